# revision 2
# baseline (speedup 1.0000x reference)
"""CURLoRA layer kernel for 8 TRN2 NeuronCores.

Computes out = x @ (W + C@U@R)^T + bias for
  x: (4, 2048, 4096) f32, W: (4096, 4096), C: (4096, 64), U: (64, 64),
  R: (64, 4096), bias: (4096,)  ->  out: (4, 2048, 4096) f32

Sharding: 8 cores = 2 token-groups x 4 output-column-groups.
Each core computes out[tg, og] = x[tg] @ (W[og] + C[og]@U@R)^T + bias[og]
independently (no collectives needed).

Per-core kernel (bf16 compute, fp32 accumulate):
  1. Build W'^T = (W_sh + C_sh@U@R)^T in SBUF as bf16 [d=128p, 32k, 1024o].
  2. Stream x tiles [128t, 4096d], cast to bf16 (SWDGE cast-DMA),
     PE-transpose into x^T tiles [128d-part, 32k, 128t].
  3. Accumulate out[t, o] over 32 k-tiles into PSUM, add bias on eviction.
"""

import sys

if "/opt/trn_rl_repo" not in sys.path:
    sys.path.insert(0, "/opt/trn_rl_repo")

import numpy as np

B, S, D = 4, 2048, 4096
O = 4096
RK = 64
T = B * S  # 8192 tokens
NT, NO = 2, 4  # token groups x out-column groups
TSH = T // NT  # 4096 tokens per core
OSH = O // NO  # 1024 out columns per core
N_CORES = 8

_CACHE = {}


def _build():
    from concourse import bacc
    import concourse.bass as bass
    import concourse.mybir as mybir
    from concourse.bass import ts
    from concourse.tile import TileContext
    from concourse.masks import make_identity

    f32 = mybir.dt.float32
    bf16 = mybir.dt.bfloat16

    nc = bacc.Bacc()
    x_ext = nc.declare_dram_parameter("x", [TSH, D], f32, isOutput=False)
    w_ext = nc.declare_dram_parameter("W", [OSH, D], f32, isOutput=False)
    c_ext = nc.declare_dram_parameter("C", [OSH, RK], f32, isOutput=False)
    u_ext = nc.declare_dram_parameter("U", [RK, RK], f32, isOutput=False)
    r_ext = nc.declare_dram_parameter("R", [RK, D], f32, isOutput=False)
    b_ext = nc.declare_dram_parameter("bias", [OSH], f32, isOutput=False)
    out_ext = nc.declare_dram_parameter("out", [TSH, OSH], f32, isOutput=True)

    NKT = D // 128  # 32 k-tiles
    NTT = TSH // 128  # 32 t-tiles per core
    NOJ = OSH // 512  # 2 o-blocks of 512

    with TileContext(nc) as tc:
        with (
            tc.tile_pool(name="const", bufs=1) as const,
            tc.tile_pool(name="wt", bufs=1) as wtp,
            tc.tile_pool(name="small", bufs=1) as small,
        ):
            ident = const.tile([128, 128], bf16)
            make_identity(nc, ident)

            # bias broadcast to all 128 partitions
            bias_sb = const.tile([128, OSH], f32)
            b_ap = b_ext[:]
            b_bc = bass.AP(
                tensor=b_ap.tensor,
                offset=b_ap.offset,
                ap=[[0, 128]] + [list(p) for p in b_ap.ap],
            )
            nc.gpsimd.dma_start(out=bias_sb[:], in_=b_bc)

            # resident W'^T: [128 d-part, 32 k-tiles, 1024 o] bf16
            wt_sb = wtp.tile([128, NKT, OSH], bf16)

            # ---------------- prologue: adapter + W'^T build ----------------
            u_sb = small.tile([RK, RK], bf16)
            nc.gpsimd.dma_start(out=u_sb[:], in_=u_ext[:])  # cast f32->bf16
            r_sb = small.tile([RK, D], bf16)
            nc.gpsimd.dma_start(out=r_sb[:], in_=r_ext[:])
            # C as [128, 8, 64] (o-tile-major)
            c_nat = small.tile([128, OSH // 128, RK], bf16)
            nc.gpsimd.dma_start(
                out=c_nat[:], in_=c_ext[:].rearrange("(j p) r -> p j r", p=128)
            )
            ut_sb = small.tile([RK, RK], bf16)
            ct_sb = small.tile([RK, OSH], bf16)
            ur_sb = small.tile([RK, D], bf16)

            with (
                tc.tile_pool(name="ps_pro", bufs=2, space="PSUM") as ps_pro,
                tc.tile_pool(name="ps_small", bufs=1, space="PSUM") as ps_small,
                tc.tile_pool(name="wpool", bufs=2) as wpool,
                tc.tile_pool(name="w2pool", bufs=2) as w2pool,
            ):
                # U^T
                ps_u = ps_small.tile([RK, RK], bf16, tag="ps_u")
                nc.tensor.transpose(ps_u[:], u_sb[:], ident[:RK, :RK])
                nc.vector.tensor_copy(out=ut_sb[:], in_=ps_u[:])

                # C^T tiles
                for j in range(OSH // 128):
                    ps_c = ps_small.tile([RK, 128], bf16, tag="ps_c")
                    nc.tensor.transpose(ps_c[:], c_nat[:, j, :], ident[:])
                    nc.vector.tensor_copy(out=ct_sb[:, ts(j, 128)], in_=ps_c[:])

                # UR = U @ R  -> [64, 4096] bf16
                for c in range(D // 512):
                    ps_ur = ps_small.tile([RK, 512], f32, tag="ps_ur")
                    nc.tensor.matmul(
                        ps_ur[:], ut_sb[:], r_sb[:, ts(c, 512)], start=True, stop=True
                    )
                    nc.vector.tensor_copy(out=ur_sb[:, ts(c, 512)], in_=ps_ur[:])

                # W' = W + C@UR per o-tile, then transpose into wt_sb
                for j in range(OSH // 128):
                    w_nat = wpool.tile([128, D], f32)
                    nc.sync.dma_start(w_nat[:], w_ext[ts(j, 128), :])
                    w2 = w2pool.tile([128, D], bf16)
                    for c in range(D // 512):
                        ps_ad = ps_pro.tile([128, 512], f32, tag="ps_ad")
                        nc.tensor.matmul(
                            ps_ad[:],
                            ct_sb[:, ts(j, 128)],
                            ur_sb[:, ts(c, 512)],
                            start=True,
                            stop=True,
                        )
                        nc.vector.tensor_add(
                            out=w2[:, ts(c, 512)],
                            in0=ps_ad[:],
                            in1=w_nat[:, ts(c, 512)],
                        )
                    for g in range(NKT // 8):
                        ps_wt = ps_pro.tile([128, 8, 128], bf16, tag="ps_wt")
                        for i in range(8):
                            nc.tensor.transpose(
                                ps_wt[:, i, :], w2[:, ts(8 * g + i, 128)], ident[:]
                            )
                        nc.vector.tensor_copy(
                            out=wt_sb[:, 8 * g : 8 * g + 8, ts(j, 128)],
                            in_=ps_wt[:],
                        )

            # ---------------- main loop: stream x ----------------
            with (
                tc.tile_pool(name="xpool", bufs=3) as xpool,
                tc.tile_pool(name="xtpool", bufs=2) as xtpool,
                tc.tile_pool(name="opool", bufs=2) as opool,
                tc.tile_pool(name="ps_xt", bufs=2, space="PSUM") as ps_xtp,
                tc.tile_pool(name="ps_main", bufs=3, space="PSUM") as ps_mainp,
            ):
                for i in range(NTT):
                    x_nat = xpool.tile([128, D], bf16)
                    nc.gpsimd.dma_start(out=x_nat[:], in_=x_ext[ts(i, 128), :])
                    xT = xtpool.tile([128, NKT, 128], bf16)
                    for g in range(NKT // 8):
                        ps_xt = ps_xtp.tile([128, 8, 128], bf16)
                        for ii in range(8):
                            nc.tensor.transpose(
                                ps_xt[:, ii, :], x_nat[:, ts(8 * g + ii, 128)], ident[:]
                            )
                        nc.vector.tensor_copy(
                            out=xT[:, 8 * g : 8 * g + 8, :], in_=ps_xt[:]
                        )
                    out_sb = opool.tile([128, OSH], f32)
                    for j in range(NOJ):
                        psm = ps_mainp.tile([128, 512], f32)
                        for k in range(NKT):
                            nc.tensor.matmul(
                                psm[:],
                                xT[:, k, :],
                                wt_sb[:, k, ts(j, 512)],
                                start=(k == 0),
                                stop=(k == NKT - 1),
                            )
                        nc.vector.tensor_add(
                            out=out_sb[:, ts(j, 512)],
                            in0=psm[:],
                            in1=bias_sb[:, ts(j, 512)],
                        )
                    nc.sync.dma_start(out_ext[ts(i, 128), :], out_sb[:])

    nc.compile()
    return nc


def kernel(x, W, C, U, R, bias):
    from concourse.bass_utils import run_bass_kernel_spmd

    x = np.ascontiguousarray(np.asarray(x, dtype=np.float32)).reshape(T, D)
    W = np.ascontiguousarray(np.asarray(W, dtype=np.float32))
    C = np.ascontiguousarray(np.asarray(C, dtype=np.float32))
    U = np.ascontiguousarray(np.asarray(U, dtype=np.float32))
    R = np.ascontiguousarray(np.asarray(R, dtype=np.float32))
    bias = np.ascontiguousarray(np.asarray(bias, dtype=np.float32))

    if "nc" not in _CACHE:
        _CACHE["nc"] = _build()
    nc = _CACHE["nc"]

    in_maps = []
    for core in range(N_CORES):
        tg, og = divmod(core, NO)
        in_maps.append(
            {
                "x": x[tg * TSH : (tg + 1) * TSH],
                "W": W[og * OSH : (og + 1) * OSH],
                "C": C[og * OSH : (og + 1) * OSH],
                "U": U,
                "R": R,
                "bias": bias[og * OSH : (og + 1) * OSH],
            }
        )

    res = run_bass_kernel_spmd(nc, in_maps, core_ids=list(range(N_CORES)))

    out = np.empty((T, O), dtype=np.float32)
    for core in range(N_CORES):
        tg, og = divmod(core, NO)
        out[tg * TSH : (tg + 1) * TSH, og * OSH : (og + 1) * OSH] = res.results[core][
            "out"
        ]
    return out.reshape(B, S, O)


# revision 5
# speedup vs baseline: 1.0321x; 1.0321x over previous
"""CURLoRA layer kernel for 8 TRN2 NeuronCores.

Computes out = x @ (W + C@U@R)^T + bias for
  x: (4, 2048, 4096) f32, W: (4096, 4096), C: (4096, 64), U: (64, 64),
  R: (64, 4096), bias: (4096,)  ->  out: (4, 2048, 4096) f32

Sharding: 8 cores = 2 token-groups x 4 output-column-groups.
Each core computes out[tg, og] = x[tg] @ (W[og] + C[og]@U@R)^T + bias[og]
independently (no collectives needed).

Per-core kernel (bf16 compute, fp32 accumulate):
  1. Build W'^T = (W_sh + C_sh@U@R)^T in SBUF as bf16 [d=128p, 32k, 1024o].
  2. Stream x tiles [128t, 4096d], cast to bf16 (SWDGE cast-DMA),
     PE-transpose into x^T tiles [128d-part, 32k, 128t].
  3. Accumulate out[t, o] over 32 k-tiles into PSUM, add bias on eviction.
A few x tiles are transposed before/through the W'-build to keep the PE
array dense early (HAM clock-gate warmup).
"""

import sys

if "/opt/trn_rl_repo" not in sys.path:
    sys.path.insert(0, "/opt/trn_rl_repo")

import numpy as np

B, S, D = 4, 2048, 4096
O = 4096
RK = 64
T = B * S  # 8192 tokens
NT, NO = 2, 4  # token groups x out-column groups
TSH = T // NT  # 4096 tokens per core
OSH = O // NO  # 1024 out columns per core
N_CORES = 8

_CACHE = {}


def _build():
    from concourse import bacc
    import concourse.bass as bass
    import concourse.mybir as mybir
    from concourse.bass import ts
    from concourse.tile import TileContext
    from concourse.masks import make_identity

    f32 = mybir.dt.float32
    bf16 = mybir.dt.bfloat16

    nc = bacc.Bacc()
    x_ext = nc.declare_dram_parameter("x", [TSH, D], f32, isOutput=False)
    w_ext = nc.declare_dram_parameter("W", [OSH, D], f32, isOutput=False)
    c_ext = nc.declare_dram_parameter("C", [OSH, RK], f32, isOutput=False)
    u_ext = nc.declare_dram_parameter("U", [RK, RK], f32, isOutput=False)
    r_ext = nc.declare_dram_parameter("R", [RK, D], f32, isOutput=False)
    b_ext = nc.declare_dram_parameter("bias", [OSH], f32, isOutput=False)
    out_ext = nc.declare_dram_parameter("out", [TSH, OSH], f32, isOutput=True)

    NKT = D // 128  # 32 k-tiles
    NTT = TSH // 128  # 32 t-tiles per core
    NOJ = OSH // 512  # 2 o-blocks of 512
    NPRE = 2  # x tiles transposed ahead of the W' build

    with TileContext(nc) as tc:
        with (
            tc.tile_pool(name="const", bufs=1) as const,
            tc.tile_pool(name="wt", bufs=1) as wtp,
            tc.tile_pool(name="small", bufs=1) as small,
            tc.tile_pool(name="xpool", bufs=3) as xpool,
            tc.tile_pool(name="xtpool", bufs=NPRE + 1) as xtpool,
            tc.tile_pool(name="opool", bufs=2) as opool,
            tc.tile_pool(name="wpool", bufs=2) as wpool,
            tc.tile_pool(name="w2pool", bufs=1) as w2pool,
            # PSUM: psT 3 + psA 3 + psS 1 = 7 banks of 8
            tc.tile_pool(name="psT", bufs=3, space="PSUM") as psT,
            tc.tile_pool(name="psA", bufs=3, space="PSUM") as psA,
            tc.tile_pool(name="psS", bufs=1, space="PSUM") as psS,
        ):
            ident = const.tile([128, 128], bf16)
            make_identity(nc, ident)

            # resident W'^T: [128 d-part, 32 k-tiles, 1024 o] bf16
            wt_sb = wtp.tile([128, NKT, OSH], bf16)
            bias_sb = const.tile([128, OSH], f32)

            def emit_x_transpose(i):
                x_nat = xpool.tile([128, D], bf16)
                nc.gpsimd.dma_start(out=x_nat[:], in_=x_ext[ts(i, 128), :])
                xT = xtpool.tile([128, NKT, 128], bf16)
                for g in range(NKT // 8):
                    ps_xt = psT.tile([128, 8, 128], bf16, tag="t")
                    for ii in range(8):
                        nc.tensor.transpose(
                            ps_xt[:, ii, :], x_nat[:, ts(8 * g + ii, 128)], ident[:]
                        )
                    nc.vector.tensor_copy(out=xT[:, 8 * g : 8 * g + 8, :], in_=ps_xt[:])
                return xT

            def emit_x_mm(i, xT):
                out_sb = opool.tile([128, OSH], f32)
                for j in range(NOJ):
                    psm = psA.tile([128, 512], f32, tag="a")
                    for k in range(NKT):
                        nc.tensor.matmul(
                            psm[:],
                            xT[:, k, :],
                            wt_sb[:, k, ts(j, 512)],
                            start=(k == 0),
                            stop=(k == NKT - 1),
                        )
                    nc.vector.tensor_add(
                        out=out_sb[:, ts(j, 512)],
                        in0=psm[:],
                        in1=bias_sb[:, ts(j, 512)],
                    )
                nc.sync.dma_start(out_ext[ts(i, 128), :], out_sb[:])

            # --- early x tiles: dense PE work while W' build streams in ---
            pre_xt = [emit_x_transpose(i) for i in range(NPRE)]

            # ---------------- adapter smalls ----------------
            u_sb = small.tile([RK, RK], bf16)
            nc.gpsimd.dma_start(out=u_sb[:], in_=u_ext[:])  # cast f32->bf16
            c_nat = small.tile([128, OSH // 128, RK], bf16)
            nc.gpsimd.dma_start(
                out=c_nat[:], in_=c_ext[:].rearrange("(j p) r -> p j r", p=128)
            )
            r_sb = small.tile([RK, D], bf16)
            nc.gpsimd.dma_start(out=r_sb[:], in_=r_ext[:])
            ut_sb = small.tile([RK, RK], bf16)
            ct_sb = small.tile([RK, OSH], bf16)
            ur_sb = small.tile([RK, D], bf16)

            # U^T
            ps_u = psS.tile([RK, 128], bf16, tag="s")
            nc.tensor.transpose(ps_u[:, :RK], u_sb[:], ident[:RK, :RK])
            nc.vector.tensor_copy(out=ut_sb[:], in_=ps_u[:, :RK])

            # C^T tiles
            for j in range(OSH // 128):
                ps_c = psS.tile([RK, 128], bf16, tag="s")
                nc.tensor.transpose(ps_c[:], c_nat[:, j, :], ident[:])
                nc.vector.tensor_copy(out=ct_sb[:, ts(j, 128)], in_=ps_c[:])

            # UR = U @ R  -> [64, 4096] bf16
            for c in range(D // 512):
                ps_ur = psA.tile([128, 512], f32, tag="a")
                nc.tensor.matmul(
                    ps_ur[:RK, :], ut_sb[:], r_sb[:, ts(c, 512)], start=True, stop=True
                )
                nc.vector.tensor_copy(out=ur_sb[:, ts(c, 512)], in_=ps_ur[:RK, :])

            # W' = W + C@UR per o-tile, then transpose into wt_sb
            for j in range(OSH // 128):
                w_nat = wpool.tile([128, D], f32)
                # alternate the two HWDGE rings for W streaming
                dma_eng = nc.sync if j % 2 == 0 else nc.scalar
                dma_eng.dma_start(w_nat[:], w_ext[ts(j, 128), :])
                w2 = w2pool.tile([128, D], bf16)
                for c in range(D // 512):
                    ps_ad = psA.tile([128, 512], f32, tag="a")
                    nc.tensor.matmul(
                        ps_ad[:],
                        ct_sb[:, ts(j, 128)],
                        ur_sb[:, ts(c, 512)],
                        start=True,
                        stop=True,
                    )
                    nc.vector.tensor_add(
                        out=w2[:, ts(c, 512)],
                        in0=ps_ad[:],
                        in1=w_nat[:, ts(c, 512)],
                    )
                for g in range(NKT // 8):
                    ps_wt = psT.tile([128, 8, 128], bf16, tag="t")
                    for i in range(8):
                        nc.tensor.transpose(
                            ps_wt[:, i, :], w2[:, ts(8 * g + i, 128)], ident[:]
                        )
                    nc.vector.tensor_copy(
                        out=wt_sb[:, 8 * g : 8 * g + 8, ts(j, 128)],
                        in_=ps_wt[:],
                    )

            # bias broadcast to all 128 partitions (needed at first eviction)
            b_ap = b_ext[:]
            b_bc = bass.AP(
                tensor=b_ap.tensor,
                offset=b_ap.offset,
                ap=[[0, 128]] + [list(p) for p in b_ap.ap],
            )
            nc.gpsimd.dma_start(out=bias_sb[:], in_=b_bc)

            # ---------------- main loop: stream x ----------------
            for i in range(NTT):
                xT = pre_xt[i] if i < NPRE else emit_x_transpose(i)
                emit_x_mm(i, xT)

    nc.compile()
    return nc


def kernel(x, W, C, U, R, bias):
    from concourse.bass_utils import run_bass_kernel_spmd

    x = np.ascontiguousarray(np.asarray(x, dtype=np.float32)).reshape(T, D)
    W = np.ascontiguousarray(np.asarray(W, dtype=np.float32))
    C = np.ascontiguousarray(np.asarray(C, dtype=np.float32))
    U = np.ascontiguousarray(np.asarray(U, dtype=np.float32))
    R = np.ascontiguousarray(np.asarray(R, dtype=np.float32))
    bias = np.ascontiguousarray(np.asarray(bias, dtype=np.float32))

    if "nc" not in _CACHE:
        _CACHE["nc"] = _build()
    nc = _CACHE["nc"]

    in_maps = []
    for core in range(N_CORES):
        tg, og = divmod(core, NO)
        in_maps.append(
            {
                "x": x[tg * TSH : (tg + 1) * TSH],
                "W": W[og * OSH : (og + 1) * OSH],
                "C": C[og * OSH : (og + 1) * OSH],
                "U": U,
                "R": R,
                "bias": bias[og * OSH : (og + 1) * OSH],
            }
        )

    res = run_bass_kernel_spmd(nc, in_maps, core_ids=list(range(N_CORES)))

    out = np.empty((T, O), dtype=np.float32)
    for core in range(N_CORES):
        tg, og = divmod(core, NO)
        out[tg * TSH : (tg + 1) * TSH, og * OSH : (og + 1) * OSH] = res.results[core][
            "out"
        ]
    return out.reshape(B, S, O)
